# revision 24
# baseline (speedup 1.0000x reference)
"""Trainium2 Bass kernel for DisplaceChannel (fp16, PE h-pass + DVE v-pass).

Math (per channel c, group f = c // 16):
  off_px  = offset[f] * 64;  off_int = round(off_px);  sub = off_px - off_int
  shifted[y, x] = x[y - dy, x - dx]  (zero outside), dy/dx = off_int
  out = depthwise 3x3 SAME conv of `shifted` with a normalized Gaussian
        kernel, separable as u (vertical) x v (horizontal) taps.

Device-side structure:
  - fp16 everywhere; host casts/transposes x to [C, H, W, B] (batch
    interleaved into the last dim) and widens y back to fp32.  The
    interleave merges (W, B) into one contiguous dim on both sides of
    every DMA: loads run at 256 B per descriptor, stores are one
    contiguous run per partition.  Tolerance is 2e-2; fp16 costs ~1e-3.
  - Horizontal pass on the PE array: per (batch, span, tap) accumulating
    diagonal matmuls (stationary = diag(v_t * u1)), 8-row chunks so each
    matmul output sits inside one PSUM bank; batch de-interleave happens
    in the stride-2 moving access pattern.
  - Vertical pass on DVE as two scalar_tensor_tensor ops per (batch,
    span) reading T straight from PSUM with the u0/u1, u2/u1 ratio
    scalars; no intermediate PSUM->SBUF copy (ACT engine left idle).
  - Stores cover only each group's non-zero output row band; the rest of
    y stays zero via bass2jax's zero-donated output buffers.

Channel groups are sorted by row window and packed 8-per-block (DP) so
the row-band union per block is small; all compute and IO follow bands.
"""

import os
import sys
from contextlib import ExitStack

import numpy as np

for _p in ("/opt/trn_rl_repo", "/root/.axon_site/_ro/trn_rl_repo"):
    if os.path.isdir(_p) and _p not in sys.path:
        sys.path.append(_p)

import concourse.bass as bass
import concourse.bacc as bacc
import concourse.mybir as mybir
import concourse.tile as tile
from concourse.bass_utils import run_bass_kernel_spmd

H = W = 64
C = 768
B = 16
N_CORES = 8
BPC = B // N_CORES          # batches per core
P = 128                     # partitions
NGRP = 48
GSZ = 16                    # channels per group
SCALE = 64.0
SIGMA = 0.5
FP32 = mybir.dt.float32
FP16 = mybir.dt.float16
MULT = mybir.AluOpType.mult
ADD = mybir.AluOpType.add
SPAN_MAX = 30               # max output rows per PSUM span (rt<=32 -> 4 banks)
WB = W * BPC                # interleaved (col, batch) width


def _geometry(offset: np.ndarray):
    """Integer shifts and separable 1-D taps per group, matching reference."""
    off_px = offset.astype(np.float32) * np.float32(SCALE)
    off_int = np.round(off_px)
    sub = off_px - off_int                      # [48, 2] (x, y)
    dx = off_int[:, 0].astype(np.int64)
    dy = off_int[:, 1].astype(np.int64)
    r = (np.arange(3, dtype=np.float32) - 1.0).astype(np.float32)
    ex = np.exp(-((r[None, :] + sub[:, 0:1]) ** 2) / (2.0 * SIGMA * SIGMA))
    ey = np.exp(-((r[None, :] + sub[:, 1:2]) ** 2) / (2.0 * SIGMA * SIGMA))
    v = ex / ex.sum(1, keepdims=True)           # [48, 3] horizontal taps
    u = ey / ey.sum(1, keepdims=True)           # [48, 3] vertical taps
    return dx, dy, v.astype(np.float32), u.astype(np.float32)


def _row_window(dyg: int):
    """Nonzero row range [r0, r1) of the shifted image for shift dy."""
    r0 = max(0, dyg)
    r1 = H + min(0, dyg)
    return r0, max(r0, r1)


def _partition_blocks(dy):
    """Sort groups by row window, then split into consecutive runs of <=8
    groups minimizing the summed union band (engine work ~ band height)."""
    order = sorted(range(NGRP), key=lambda g: _row_window(int(dy[g])))
    r0s = [_row_window(int(dy[g]))[0] for g in order]
    r1s = [_row_window(int(dy[g]))[1] for g in order]
    INF = float("inf")
    best = [INF] * (NGRP + 1)
    prev = [0] * (NGRP + 1)
    best[0] = 0.0
    for e in range(1, NGRP + 1):
        for s in range(max(0, e - 8), e):
            band = max(r1s[s:e]) - min(r0s[s:e])
            cost = best[s] + band * 500.0 + 3000.0
            if cost < best[e]:
                best[e] = cost
                prev[e] = s
    cuts = []
    e = NGRP
    while e > 0:
        s = prev[e]
        cuts.append((s, e))
        e = s
    blocks = []
    for s, e in reversed(cuts):
        blocks.append((order[s:e], min(r0s[s:e]), max(r1s[s:e])))
    # process small bands first so the startup (memset+load) bubble is short
    blocks.sort(key=lambda blk: blk[2] - blk[1])
    return blocks


def _spans(v0: int, v1: int):
    """Split output rows [v0, v1) into near-equal spans of <= SPAN_MAX."""
    nv = v1 - v0
    n = (nv + SPAN_MAX - 1) // SPAN_MAX
    out = []
    a = v0
    for i in range(n - 1):
        sp = min(22, nv - (a - v0) - 1)
        out.append((a, a + sp))
        a += sp
    out.append((a, v1))
    return out


def _build(offset: np.ndarray) -> bass.Bass:
    dx, dy, v, u = _geometry(offset)
    blocks = _partition_blocks(dy)
    nblk = len(blocks)

    # Stationary tap matrices diag(v_t * u1) per (block, tap), partition-major
    # so the upload is a straight copy; rat[p, bi] = (u0/u1, u2/u1).
    stat_np = np.zeros((P, nblk, 3, P), dtype=np.float16)
    rat_np = np.zeros((P, nblk, 5), dtype=np.float32)
    for bi, (groups, _, _) in enumerate(blocks):
        for gl, g in enumerate(groups):
            sl = slice(gl * GSZ, (gl + 1) * GSZ)
            for t in range(3):
                stat_np[sl, bi, t, sl] = np.eye(GSZ, dtype=np.float16) * \
                    np.float16(v[g, t] * u[g, 1])
            rat_np[sl, bi, 0] = u[g, 0] / u[g, 1]
            rat_np[sl, bi, 1] = u[g, 2] / u[g, 1]
            rat_np[sl, bi, 2:5] = v[g, :] * u[g, 1]

    nc = bacc.Bacc("TRN2", target_bir_lowering=False, debug=False)
    x_in = nc.dram_tensor("x", [C, H, W, BPC], FP16, kind="ExternalInput")
    y_out = nc.dram_tensor("y", [C, H, BPC, W], FP16, kind="ExternalOutput")
    stat_d = nc.inline_tensor(stat_np, name="stat")
    rat_d = nc.inline_tensor(rat_np, name="rat")

    with tile.TileContext(nc) as tc, ExitStack() as ctx:
        c_pool = ctx.enter_context(tc.tile_pool(name="const", bufs=1))
        s_pool = ctx.enter_context(tc.tile_pool(name="s", bufs=1))
        t_pool = ctx.enter_context(
            tc.tile_pool(name="tpsum", bufs=2, space="PSUM"))
        m_pool = ctx.enter_context(tc.tile_pool(name="m", bufs=3))
        t2_pool = ctx.enter_context(tc.tile_pool(name="t2", bufs=2))
        o_pool = ctx.enter_context(tc.tile_pool(name="o", bufs=3))

        stat = c_pool.tile([P, nblk, 3, P], FP16, name="stat", tag="stat")
        rat = c_pool.tile([P, nblk, 5], FP32, name="rat", tag="rat")
        nc.sync.dma_start(stat[:], stat_d[:])
        nc.sync.dma_start(rat[:], rat_d[:])

        # Per-block S tiles: shifted-image band, batch-interleaved columns.
        # Virtual rows [r0-2, r1+2); virtual col c / batch b at 2*(c+1)+b
        # (cols -1 and 64 are the conv zero border).  Pre-zeroed; loads fill
        # only valid windows.
        qload = {id(nc.sync): 0.0, id(nc.scalar): 0.0, id(nc.gpsimd): 0.0}
        qcost = {id(nc.sync): 4.0, id(nc.scalar): 4.0, id(nc.gpsimd): 0.55}
        qengs = [nc.sync, nc.scalar, nc.gpsimd]
        emit_order = list(range(nblk))
        if nblk > 1:
            emit_order[0], emit_order[1] = emit_order[1], emit_order[0]
        s_tiles_map = {}
        for bi in emit_order:
            groups, r0, r1 = blocks[bi]
            hh = (r1 - r0) + 4
            S = s_pool.tile([P, hh, 2 * (W + 2)], FP16, name=f"S{bi}",
                            tag=f"S{bi}")
            s_tiles_map[bi] = S
            # split memsets across engines: a single serial chain on Pool
            # would delay later blocks' loads and starve the PE
            if bi == 0:
                nc.vector.memset(S[:], 0.0)
            elif bi in (1, 2):
                nc.scalar.memzero(S[:])
            else:
                nc.gpsimd.memset(S[:], 0.0)
            for gl, g in enumerate(groups):
                dyg, dxg = int(dy[g]), int(dx[g])
                gr0, gr1 = _row_window(dyg)
                ny, nx = gr1 - gr0, W - abs(dxg)
                if ny <= 0 or nx <= 0:
                    continue
                ys = max(0, -dyg)
                xs, xd = max(0, -dxg), max(0, dxg)
                ch0 = g * GSZ
                # queue-balance: HWDGE ~4ns/desc serial per queue, SWDGE
                # ~0.34ns/desc + ~1us fixed; pick the least-loaded queue
                eng = min(qengs, key=lambda e: qload[id(e)])
                qload[id(eng)] += (16 * ny) * qcost[id(eng)] + \
                    (1000.0 if eng is nc.gpsimd else 600.0)
                eng.dma_start(
                    S[gl * GSZ:(gl + 1) * GSZ,
                      gr0 - (r0 - 2):gr0 - (r0 - 2) + ny,
                      2 * (xd + 1):2 * (xd + 1) + 2 * nx],
                    x_in[ch0:ch0 + GSZ, ys:ys + ny, xs:xs + nx, :],
                )

        # blocks whose horizontal pass runs on DVE/ACT instead of the PE,
        # balancing tensor-engine time against vector-engine time
        # first block's horizontal pass runs on DVE/ACT instead of the PE:
        # sheds ~8us of tensor-engine time onto the vector engine's slack
        # and gives DVE work during the PE's startup window
        dve_route = {0}

        for bi, (groups, r0, r1) in enumerate(blocks):
            S = s_tiles_map[bi]
            v0 = max(r0 - 1, 0)
            v1 = min(r1 + 1, H)
            O = o_pool.tile([P, H, BPC, W], FP16, name="O", tag="O")
            for (o0, o1) in _spans(v0, v1):
                sp = o1 - o0
                rt = sp + 2                       # T rows [o0-1, o1+1)
                sr0 = (o0 - 1) - (r0 - 2)         # T start row within S
                if bi in dve_route:
                    # h-pass on ACT (center, scaled copy) + DVE (side taps),
                    # v-pass on DVE reading the SBUF T2 directly
                    T2 = t2_pool.tile([P, 32, BPC, W], FP16, name="T2",
                                      tag="T2")
                    for b in range(BPC):
                        nc.scalar.mul(
                            T2[:, 0:rt, b, :],
                            S[:, sr0:sr0 + rt, 2 + b:2 + b + 2 * W - 1:2],
                            rat[:, bi, 3:4])
                        nc.vector.scalar_tensor_tensor(
                            T2[:, 0:rt, b, :],
                            S[:, sr0:sr0 + rt, b:b + 2 * W - 1:2],
                            rat[:, bi, 2:3], T2[:, 0:rt, b, :], MULT, ADD)
                        nc.vector.scalar_tensor_tensor(
                            T2[:, 0:rt, b, :],
                            S[:, sr0:sr0 + rt, 4 + b:4 + b + 2 * W - 1:2],
                            rat[:, bi, 4:5], T2[:, 0:rt, b, :], MULT, ADD)
                        M = m_pool.tile([P, SPAN_MAX, W], FP32, name="M",
                                        tag="M")
                        nc.vector.scalar_tensor_tensor(
                            M[:, 0:sp, :], T2[:, 0:sp, b, :],
                            rat[:, bi, 0:1], T2[:, 1:sp + 1, b, :], MULT, ADD)
                        nc.vector.scalar_tensor_tensor(
                            O[:, o0 - v0:o1 - v0, b, :],
                            T2[:, 2:sp + 2, b, :], rat[:, bi, 1:2],
                            M[:, 0:sp, :], MULT, ADD)
                    continue
                for b in range(BPC):
                    # fixed 32-row (4-bank) psum tile; 8-row chunks = banks
                    T = t_pool.tile([P, 32, W], FP32, name="T", tag="T")
                    for t in range(3):
                        for c0 in range(0, rt, 8):
                            rows = min(8, rt - c0)
                            nc.tensor.matmul(
                                T[:, c0:c0 + rows, :],
                                stat[:, bi, t, :],
                                S[:, sr0 + c0:sr0 + c0 + rows,
                                  2 * t + b:2 * t + b + 2 * W - 1:2],
                                start=(t == 0), stop=(t == 2),
                            )
                    # vertical taps: DVE may read only one PSUM operand per
                    # instruction, so ACT first copies the center rows to
                    # SBUF, then two chained scalar_tensor_tensor ops do
                    #   M = r0*T[y-1] + Cc;  O = r2*T[y+1] + M
                    Cc = m_pool.tile([P, SPAN_MAX, W], FP32, name="Cc",
                                     tag="Cc")
                    M = m_pool.tile([P, SPAN_MAX, W], FP32, name="M", tag="M")
                    nc.scalar.copy(Cc[:, 0:sp, :], T[:, 1:sp + 1, :])
                    nc.vector.scalar_tensor_tensor(
                        M[:, 0:sp, :], T[:, 0:sp, :], rat[:, bi, 0:1],
                        Cc[:, 0:sp, :], MULT, ADD)
                    nc.vector.scalar_tensor_tensor(
                        O[:, o0 - v0:o1 - v0, b, :],
                        T[:, 2:sp + 2, :], rat[:, bi, 1:2],
                        M[:, 0:sp, :], MULT, ADD)
            # stores per group, restricted to its own non-zero output rows;
            # the rest of y stays zero via the donated zero output buffer
            for gl, g in enumerate(groups):
                gr0, gr1 = _row_window(int(dy[g]))
                if gr1 <= gr0 or W - abs(int(dx[g])) <= 0:
                    continue
                w0 = max(gr0 - 1, 0)
                w1 = min(gr1 + 1, H)
                ch0 = g * GSZ
                nc.sync.dma_start(
                    y_out[ch0:ch0 + GSZ, w0:w1, :, :],
                    O[gl * GSZ:(gl + 1) * GSZ, w0 - v0:w1 - v0, :, :],
                )

    nc.compile()
    return nc


def _run(x: np.ndarray, offset: np.ndarray, trace: bool = False):
    offset = np.ascontiguousarray(offset, dtype=np.float32)
    nc = _build(offset)
    in_maps = []
    for k in range(N_CORES):
        xk = x[k * BPC:(k + 1) * BPC].transpose(1, 2, 3, 0)  # [C, H, W, B]
        in_maps.append({"x": np.ascontiguousarray(xk, dtype=np.float16)})
    res = run_bass_kernel_spmd(
        nc, in_maps, core_ids=list(range(N_CORES)), trace=trace
    )
    outs = []
    for k in range(N_CORES):
        yk = np.asarray(res.results[k]["y"])      # [C, H, BPC, W] fp16
        outs.append(yk.transpose(2, 0, 1, 3).astype(np.float32))
    return np.concatenate(outs, axis=0), res


def kernel(x: np.ndarray, offset: np.ndarray) -> np.ndarray:
    return _run(x, offset)[0]
